# revision 21
# baseline (speedup 1.0000x reference)
"""Causal self-attention with RoPE on 8 Trainium2 NeuronCores.

Problem (hardcoded): B=2, S=2048, E=2048, H=16 heads, D=128 head dim.
  qkv = x @ W_qkv.T ; RoPE(q, k) ; causal softmax attention ; out @ W_out.T

Sharding: tensor-parallel over heads. Each of the 8 cores handles 2 heads
for both batches and produces a partial output projection (row-sharded
W_out); the host sums the 8 partial outputs.

Key device-side choices (v2):
 - fp16 everywhere off-PSUM (10-bit mantissa beats bf16; same PE speed).
   The 1/sqrt(D) score scale is folded into W_q on the host, so q and k
   share one cos/sin table pair.
 - qkv: w-stationary k-major matmuls for q/k (feature-major output, which
   RoPE and the scores matmul want); v is produced with x-stationary
   matmuls directly in [token, head_dim] layout - no PE transposes.
 - Scores are computed transposed (scoresT [k, q]) in PAIRS of 128-k-tiles
   into a [128, 2, 512] PSUM tile so one exp activation covers 1024
   columns (the scalar engine's fixed per-instruction overhead was the
   attention bottleneck). Diagonal-band tiles stay ragged with separate
   exps.
 - Softmax denominators: exp tiles are merged with fp16 vector adds into
   one [128, 512] tile per (head, q-block), then a single gpsimd
   partition_all_reduce produces the broadcast column sums - no PE
   ones-matmuls, no DRAM-bounce broadcast, and only 8 PSUM banks needed.
 - The output projection of unit u-1 is interleaved chain-by-chain into
   unit u's score/PV stream so the PE never sits idle while the scalar
   engine works through exps.
"""

import math
from contextlib import ExitStack

import numpy as np
import ml_dtypes

import concourse.bass as bass
import concourse.mybir as mybir
import concourse.tile as tile
from concourse import bacc
from concourse import bass_isa
from concourse.bass_utils import run_bass_kernel_spmd

F16 = mybir.dt.float16
F32 = mybir.dt.float32
P = 128

# problem config
B, S, E = 2, 2048, 2048
H, D = 16, 128
N_CORES = 8
HPC = H // N_CORES  # heads per core = 2


def build_nc(b=B, s=S, e=E, hpc=HPC):
    T = b * s            # total tokens
    NT = T // 512        # 512-token blocks
    KE = e // P          # contraction tiles for the qkv projection
    QT = s // 512        # 512-wide q blocks per batch
    KT = s // P          # 128-wide k blocks per batch
    ME = e // P          # output-embedding tiles
    MQK = 2 * hpc        # q/k feature tiles per core (q0,q1,k0,k1)
    VW = hpc * P         # v feature width (256)

    nc = bacc.Bacc("TRN2", target_bir_lowering=False, debug=False)

    xT = nc.dram_tensor("xT", [P, KE, T], F16, kind="ExternalInput").ap()
    wqkv = nc.dram_tensor("wqkv", [P, KE, 6 * P], F16, kind="ExternalInput").ap()
    wo = nc.dram_tensor("wo", [P, hpc, e], F16, kind="ExternalInput").ap()
    cos_d = nc.dram_tensor("cos_d", [P, s], F16, kind="ExternalInput").ap()
    sin_d = nc.dram_tensor("sin_d", [P, s], F16, kind="ExternalInput").ap()
    bandmask = nc.dram_tensor("bandmask", [P, P], F16, kind="ExternalInput").ap()
    outT = nc.dram_tensor("outT", [e, T], F16, kind="ExternalOutput").ap()

    with tile.TileContext(nc) as tc, ExitStack() as ctx:
        persist = ctx.enter_context(tc.tile_pool(name="persist", bufs=1))
        attn_pool = ctx.enter_context(tc.tile_pool(name="attnstore", bufs=1))
        # phase-2 SBUF working pools created before phase-1 pools so their
        # addresses don't overlap phase-1's.
        exp_pool = ctx.enter_context(tc.tile_pool(name="expp", bufs=6))
        acc_pool = ctx.enter_context(tc.tile_pool(name="accp", bufs=2))
        rb_pool = ctx.enter_context(tc.tile_pool(name="rbp", bufs=2))
        osb_pool = ctx.enter_context(tc.tile_pool(name="osbp", bufs=4))
        qk_pool = tc.alloc_tile_pool(name="qkvstore", bufs=1)

        mask_sb = persist.tile([P, P], F16)
        wo_sb = persist.tile([P, hpc, e], F16)
        ones_mat = persist.tile([P, P], F16)
        nc.vector.memset(ones_mat, 1.0)

        attn_sb = [attn_pool.tile([P, T], F16, name=f"attnsb{h}") for h in range(hpc)]
        qk_sb = [qk_pool.tile([P, T], F16, name=f"qksb{i}") for i in range(MQK)]
        v_sb = qk_pool.tile([P, T // P, VW], F16, name="vsb")

        # ---- phase 1: qkv projection + RoPE (v lands pre-transposed) ----
        with ExitStack() as p1:
            wpool = p1.enter_context(tc.tile_pool(name="wq", bufs=1))
            xpool = p1.enter_context(tc.tile_pool(name="xs", bufs=3))
            trig_pool = p1.enter_context(tc.tile_pool(name="trig", bufs=1))
            rope_pool = p1.enter_context(tc.tile_pool(name="rope", bufs=3))
            qkv_ps = p1.enter_context(tc.tile_pool(name="qkvps", bufs=6, space="PSUM"))
            v_ps = p1.enter_context(tc.tile_pool(name="vps", bufs=2, space="PSUM"))

            # DMA issue order matches consumption: per-k q/k weights plus the
            # first x block first (first matmul can start ~1us in), then v
            # weights, trig, wo, mask.
            w_sb = wpool.tile([P, KE, 6 * P], F16)
            x_tiles = [None] * NT
            x_tiles[0] = xpool.tile([P, KE, 512], F16, name="x_sb")
            trig = {nm: trig_pool.tile([P, s], F16, name=nm + "_sb")
                    for nm in ("cos", "sin")}
            for k in range(KE):
                nc.sync.dma_start(w_sb[:, k, 0:MQK * P], wqkv[:, k, 0:MQK * P])
                nc.sync.dma_start(x_tiles[0][:, k, :], xT[:, k, 0:512])
                if k == 7:
                    nc.sync.dma_start(trig["cos"], cos_d)
                if k == 11:
                    nc.sync.dma_start(trig["sin"], sin_d)
            for k in range(KE):
                nc.sync.dma_start(w_sb[:, k, MQK * P:6 * P],
                                  wqkv[:, k, MQK * P:6 * P])

            for n in range(NT):
                x_sb = x_tiles[n]
                # prefetch next block (k-ascending order)
                if n + 1 < NT:
                    x_tiles[n + 1] = xpool.tile([P, KE, 512], F16, name="x_sb")
                    for k in range(KE):
                        nc.sync.dma_start(x_tiles[n + 1][:, k, :],
                                          xT[:, k, (n + 1) * 512:(n + 2) * 512])
                if n == 2:  # wo/mask are phase-2-only; keep the head of the
                    nc.sync.dma_start(mask_sb, bandmask)  # queue for x/w/trig
                    nc.sync.dma_start(wo_sb, wo)
                s0 = (n % QT) * 512  # position offset within the batch
                # q/k: k-major across the 4 feature tiles
                ps_m = [qkv_ps.tile([P, 512], F32, name="qkps") for _ in range(MQK)]
                for k in range(KE):
                    for m in range(MQK):
                        nc.tensor.matmul(
                            ps_m[m], w_sb[:, k, m * P:(m + 1) * P], x_sb[:, k, :],
                            start=(k == 0), stop=(k == KE - 1),
                        )
                c_t = trig["cos"][:, s0:s0 + 512]
                s_t = trig["sin"][:, s0:s0 + 512]
                for m in range(MQK):
                    ps = ps_m[m]
                    raw = rope_pool.tile([P, 512], F16, name="raw")
                    nc.scalar.copy(out=raw, in_=ps)
                    shuf = rope_pool.tile([P, 512], F16, name="shuf")
                    nc.vector.tensor_copy(out=shuf[0:64], in_=raw[64:128])
                    nc.vector.tensor_copy(out=shuf[64:128], in_=raw[0:64])
                    t1 = rope_pool.tile([P, 512], F16, name="t1")
                    nc.vector.tensor_mul(t1, raw, c_t)
                    nc.vector.tensor_mul(shuf, shuf, s_t)
                    nc.vector.tensor_add(qk_sb[m][:, n * 512:(n + 1) * 512],
                                         t1, shuf)
                # v: x-stationary, output [token, head_dim] directly
                for tt in range(4):
                    psv = v_ps.tile([P, VW], F32, name="vps")
                    for k in range(KE):
                        nc.tensor.matmul(
                            psv, x_sb[:, k, tt * P:(tt + 1) * P],
                            w_sb[:, k, MQK * P:6 * P],
                            start=(k == 0), stop=(k == KE - 1),
                        )
                    nc.scalar.copy(out=v_sb[:, n * 4 + tt, :], in_=psv)

        # ---- phase 2: attention with interleaved output projection ----
        with ExitStack() as p2:
            sc_ps = p2.enter_context(tc.tile_pool(name="scps", bufs=2, space="PSUM"))
            att_ps = p2.enter_context(tc.tile_pool(name="attps", bufs=2, space="PSUM"))
            out_ps = p2.enter_context(tc.tile_pool(name="outps", bufs=2, space="PSUM"))

            def emit_op_chain(nt, mt):
                ops = out_ps.tile([P, 512], F32, name="ops")
                for h in range(hpc):
                    nc.tensor.matmul(
                        ops, wo_sb[:, h, mt * P:(mt + 1) * P],
                        attn_sb[h][:, nt * 512:(nt + 1) * 512],
                        start=(h == 0), stop=(h == hpc - 1),
                    )
                osb = osb_pool.tile([P, 512], F16, name="osb")
                if mt % 2 == 0:
                    nc.scalar.copy(out=osb, in_=ops)
                else:
                    nc.vector.tensor_copy(out=osb, in_=ops)
                nc.sync.dma_start(
                    outT[mt * P:(mt + 1) * P, nt * 512:(nt + 1) * 512], osb)

            pending_chains = []   # (nt, mt) outproj chains of the previous unit

            units = [(bb, qt) for bb in range(b) for qt in range(QT)]
            for bb, qt in units:
                nk = 4 * (qt + 1)
                npairs = 2 * qt + 2
                total_steps = hpc * npairs
                step = 0
                for h in range(hpc):
                    q_sl = qk_sb[h][:, bb * s + qt * 512: bb * s + (qt + 1) * 512]
                    k_store = qk_sb[hpc + h]
                    att = att_ps.tile([P, 512], F32, name="att")
                    acc = acc_pool.tile([P, 512], F16, name="acc")
                    for i in range(npairs):
                        # interleave the previous unit's outproj chains so
                        # the PE has filler while the scalar engine works
                        # through the exps
                        steps_left = total_steps - step
                        if pending_chains:
                            n_emit = -(-len(pending_chains) // steps_left)
                            for _ in range(n_emit):
                                emit_op_chain(*pending_chains.pop(0))
                        step += 1

                        diag = i >= 2 * qt
                        sp = sc_ps.tile([P, 2, 512], F32, name="sp")
                        e_t = exp_pool.tile([P, 2, 512], F16, name="e_t")
                        halves = []
                        for half in range(2):
                            kt = 2 * i + half
                            j = kt - 4 * qt
                            off = max(0, P * j)
                            w_q = 512 - off
                            halves.append((kt, off, w_q))
                            nc.tensor.matmul(
                                sp[:, half, 0:w_q],
                                k_store[:, bb * s + kt * P: bb * s + (kt + 1) * P],
                                q_sl[:, off:512], start=True, stop=True,
                            )
                        if not diag:
                            nc.scalar.activation(
                                e_t, sp, mybir.ActivationFunctionType.Exp)
                        else:
                            for half, (kt, off, w_q) in enumerate(halves):
                                nc.scalar.activation(
                                    e_t[:, half, 0:w_q], sp[:, half, 0:w_q],
                                    mybir.ActivationFunctionType.Exp)
                                nc.vector.tensor_mul(
                                    e_t[:, half, 0:P], e_t[:, half, 0:P],
                                    mask_sb)
                        for half, (kt, off, w_q) in enumerate(halves):
                            nc.tensor.matmul(
                                att[:, off:512],
                                v_sb[:, bb * KT + kt, h * P:(h + 1) * P],
                                e_t[:, half, 0:w_q],
                                start=(kt == 0), stop=(kt == nk - 1),
                            )
                        # fp16 merge adds for the softmax denominator
                        if i == 0:
                            if qt > 0:
                                nc.vector.tensor_add(acc, e_t[:, 0, :],
                                                     e_t[:, 1, :])
                            else:  # first pair is the (512, 384) diagonal pair
                                nc.vector.tensor_copy(out=acc[:, 0:P],
                                                      in_=e_t[:, 0, 0:P])
                                nc.vector.tensor_add(acc[:, P:512],
                                                     e_t[:, 0, P:512],
                                                     e_t[:, 1, 0:384])
                        elif not diag:
                            nc.vector.tensor_add(acc, acc, e_t[:, 0, :])
                            nc.vector.tensor_add(acc, acc, e_t[:, 1, :])
                        elif i == 2 * qt:  # (512, 384) diagonal pair, qt > 0
                            nc.vector.tensor_add(acc, acc, e_t[:, 0, :])
                            nc.vector.tensor_add(acc[:, P:512], acc[:, P:512],
                                                 e_t[:, 1, 0:384])
                        else:  # (256, 128) diagonal pair
                            nc.vector.tensor_add(acc[:, 2 * P:512],
                                                 acc[:, 2 * P:512],
                                                 e_t[:, 0, 0:2 * P])
                            nc.vector.tensor_add(acc[:, 3 * P:512],
                                                 acc[:, 3 * P:512],
                                                 e_t[:, 1, 0:P])
                    # softmax denominator: acc is already fully merged, so a
                    # single matmul against an all-ones [128,128] lhsT gives
                    # the column sums replicated into every partition - the
                    # broadcast is free.
                    sm = out_ps.tile([P, 512], F32, name="ops")
                    nc.tensor.matmul(sm, ones_mat, acc, start=True, stop=True)
                    rb = rb_pool.tile([P, 512], F32, name="rb")
                    nc.vector.reciprocal_approx_fast(out=rb, in_=sm)
                    nc.vector.tensor_tensor(
                        attn_sb[h][:, bb * s + qt * 512: bb * s + (qt + 1) * 512],
                        att, rb, mybir.AluOpType.mult)
                assert not pending_chains
                nt = bb * QT + qt
                pending_chains = [(nt, mt) for mt in range(ME)]
            for ch in pending_chains:
                emit_op_chain(*ch)

        qk_pool.release()

    nc.compile()
    return nc


def make_common_inputs(x, b=B, s=S, e=E):
    """Inputs identical on every core: xT, trig tables, causal band mask."""
    T = b * s
    KE = e // P
    xflat = np.ascontiguousarray(x.reshape(T, e).T)        # [E, T] f32
    xT = np.ascontiguousarray(
        xflat.reshape(KE, P, T).transpose(1, 0, 2)).astype(np.float16)

    inv_freq = (1.0 / (10000.0 ** (np.arange(0, D, 2, dtype=np.float32) / D)))
    t = np.arange(s, dtype=np.float32)
    freqs = np.outer(t, inv_freq)                           # [S, 64]
    cos = np.cos(freqs).astype(np.float32)
    sin = np.sin(freqs).astype(np.float32)
    cosT = np.concatenate([cos, cos], axis=1).T             # [128, S]
    sinT = np.concatenate([sin, sin], axis=1).T
    sgn = np.where(np.arange(D) < D // 2, -1.0, 1.0).astype(np.float32)[:, None]
    cos_d = np.ascontiguousarray(cosT).astype(np.float16)
    sin_d = np.ascontiguousarray(sinT * sgn).astype(np.float16)

    r = np.arange(P)[:, None]
    cc = np.arange(P)[None, :]
    bandmask = (cc >= r).astype(np.float16)

    return {"xT": xT, "cos_d": cos_d, "sin_d": sin_d, "bandmask": bandmask}


def make_core_inputs(W_qkv, W_out, core, b=B, s=S, e=E, hpc=HPC):
    """Per-core column-sharded W_qkv (as lhsT tiles) and row-sharded W_out.

    The 1/sqrt(D) attention scale is folded into the q rows.
    """
    KE = e // P
    heads = [core * hpc + i for i in range(hpc)]
    scale = 1.0 / math.sqrt(D)
    rows = []
    for base, rs in ((0, scale), (e, 1.0), (2 * e, 1.0)):  # q, k, v rows
        for h in heads:
            rows.append(W_qkv[base + h * D: base + (h + 1) * D] * rs)
    Wc = np.concatenate(rows, axis=0)                       # [6*128, E]
    WcT = np.ascontiguousarray(Wc.T)                        # [E, 6*128]
    wqkv = np.ascontiguousarray(
        WcT.reshape(KE, P, 6 * P).transpose(1, 0, 2)).astype(np.float16)

    wo = np.stack(
        [np.ascontiguousarray(W_out[:, h * D:(h + 1) * D].T) for h in heads],
        axis=1)                                             # [128, hpc, E]
    wo = np.ascontiguousarray(wo).astype(np.float16)
    return {"wqkv": wqkv, "wo": wo}


_NC_CACHE = {}


def get_nc():
    key = (B, S, E, HPC)
    if key not in _NC_CACHE:
        _NC_CACHE[key] = build_nc()
    return _NC_CACHE[key]


def kernel(x, W_qkv, W_out):
    x = np.asarray(x, dtype=np.float32)
    W_qkv = np.asarray(W_qkv, dtype=np.float32)
    W_out = np.asarray(W_out, dtype=np.float32)

    nc = get_nc()
    common = make_common_inputs(x)
    in_maps = [dict(common, **make_core_inputs(W_qkv, W_out, c))
               for c in range(N_CORES)]
    res = run_bass_kernel_spmd(nc, in_maps, list(range(N_CORES)))
    total = res.results[0]["outT"].astype(np.float32)
    for c in range(1, N_CORES):
        total = total + res.results[c]["outT"].astype(np.float32)
    return np.ascontiguousarray(total.T).reshape(B, S, E).astype(np.float32)
